# revision 8
# baseline (speedup 1.0000x reference)
"""GCN forward pass on 8 Trainium2 NeuronCores (Bass/Tile).

Per conv layer l (h0 = x):
  z_l = (dinv * h_l) @ W_l            -- PE matmuls on transposed chunks
  AllGather z_l shards -> z_full      -- DRAM, bf16
  msgs = z_full[src]                  -- indirect-DMA row gather (bitcast f32)
  agg = scatter_add(dinv[dst]*msgs)   -- narrow one-hot matmuls into PSUM
  h_{l+1} = relu(BN(agg))             -- stats via reduce + tiny AllReduce
Then global mean pool (one-hot matmul, 1/count folded in) + final linear.

Nodes are sharded contiguously (graph-aligned) across the 8 cores; gather
indices and one-hot value tiles are precomputed on CPU. The NEFF is SPMD:
per-chunk shapes are uniform across cores (value-0 pads).
"""
import sys
sys.path.insert(0, "/opt/trn_rl_repo")
import numpy as np
import ml_dtypes

BF16 = ml_dtypes.bfloat16
NCORES = 8
F = 64
C = 6
EPS = 1e-5
WIN = 32          # dst-window width (one-hot M dim)


# ----------------------------------------------------------------------------
# CPU preprocessing
# ----------------------------------------------------------------------------
def prep(x, edge_index, batch, G):
    N = x.shape[0]
    src = np.asarray(edge_index[0], dtype=np.int64)
    dst = np.asarray(edge_index[1], dtype=np.int64)
    batch = np.asarray(batch, dtype=np.int64)

    deg = np.bincount(dst, minlength=N).astype(np.float32) + 1.0
    dinv = 1.0 / np.sqrt(deg)

    # --- shard graphs to cores, balancing edges + nodes ---
    n_per_g = np.bincount(batch, minlength=G)
    e_per_g = np.bincount(batch[dst], minlength=G)
    cost = e_per_g + 2.0 * n_per_g + 1.0
    cum = np.cumsum(cost)
    total = cum[-1]
    bounds = [0]
    for k in range(1, NCORES):
        bounds.append(int(np.searchsorted(cum, total * k / NCORES)))
    bounds.append(G)
    for k in range(1, NCORES + 1):
        if bounds[k] <= bounds[k - 1]:
            bounds[k] = min(G - (NCORES - k), bounds[k - 1] + 1)
    g_start = np.array(bounds[:-1])
    g_end = np.array(bounds[1:])

    gnode_start = np.searchsorted(batch, g_start)
    gnode_end = np.searchsorted(batch, g_end)
    n_c = gnode_end - gnode_start
    g_c = g_end - g_start
    NSH = int(np.ceil(max(n_c.max(), 1) / 128) * 128)
    GSH = int(g_c.max())
    assert GSH <= 128, f"GSH={GSH} > 128"
    NPCH = NSH // 128
    NWIN = NSH // WIN

    shard_of = np.empty(N, np.int32)
    slot_of = np.empty(N, np.int64)
    for c in range(NCORES):
        s, e = gnode_start[c], gnode_end[c]
        shard_of[s:e] = c
        slot_of[s:e] = np.arange(e - s)
    zrow_of = shard_of.astype(np.int64) * NSH + slot_of

    # --- edge lists (incl. self-loops), grouped by dst shard, sorted by slot
    all_src = np.concatenate([src, np.arange(N, dtype=np.int64)])
    all_dst = np.concatenate([dst, np.arange(N, dtype=np.int64)])
    e_shard = shard_of[all_dst]
    e_slot = slot_of[all_dst]
    e_val = dinv[all_dst]
    e_row = zrow_of[all_src]

    per_core = []
    wcounts = np.zeros((NCORES, NWIN), np.int64)
    for c in range(NCORES):
        m = e_shard == c
        sl = e_slot[m]
        o = np.argsort(sl, kind="stable")
        sl = sl[o]
        rows = e_row[m][o]
        vals = e_val[m][o]
        wins = sl // WIN
        wcounts[c] = np.bincount(wins, minlength=NWIN)
        per_core.append((sl, rows, vals))

    # per-window chunk budget = max over cores (shared SPMD program)
    budget = np.maximum(1, np.ceil(wcounts.max(0) / 128.0)).astype(np.int64)
    chunk_win = np.repeat(np.arange(NWIN), budget)
    NCH = int(budget.sum())
    w_first = np.concatenate([[0], np.cumsum(budget)[:-1]])

    idx_np = np.zeros((NCORES, 128, NCH), np.int32)
    hot_np = np.zeros((NCORES, 128, NCH, WIN), BF16)
    for c in range(NCORES):
        sl, rows, vals = per_core[c]
        estart = 0
        ecounts = wcounts[c]
        idx_np[c, :, :] = c * NSH
        for w in range(NWIN):
            ne = int(ecounts[w])
            ch0 = w_first[w]
            for k in range(int(np.ceil(ne / 128.0))):
                a = estart + k * 128
                b = min(estart + ne, a + 128)
                kk = b - a
                ch = ch0 + k
                idx_np[c, :kk, ch] = rows[a:b]
                hot_np[c, np.arange(kk), ch, (sl[a:b] % WIN)] = vals[a:b]
            estart += ne

    # --- per-shard dense inputs ---
    xT_np = np.zeros((NCORES, F, NSH), BF16)
    dinvR_np = np.zeros((NCORES, F, NSH), BF16)
    pool_np = np.zeros((NCORES, 128, NPCH, 128), BF16)
    cnt = np.bincount(batch, minlength=G).astype(np.float32)
    inv_cnt = 1.0 / np.maximum(cnt, 1.0)
    x = np.asarray(x, np.float32)
    for c in range(NCORES):
        s, e = gnode_start[c], gnode_end[c]
        n = e - s
        xT_np[c, :, :n] = (dinv[s:e, None] * x[s:e]).T.astype(BF16)
        dinvR_np[c, :, :n] = np.broadcast_to(
            dinv[s:e].astype(BF16)[None, :], (F, n))
        lg = (batch[s:e] - g_start[c]).astype(np.int64)
        slots = np.arange(n)
        pool_np[c, slots % 128, slots // 128, lg] = \
            inv_cnt[batch[s:e]].astype(BF16)
    pool_np = pool_np[:, :, :, :GSH].copy()

    cfg = dict(NSH=NSH, GSH=GSH, NPCH=NPCH, NWIN=NWIN, NCH=NCH,
               N=N, chunk_win=tuple(chunk_win.tolist()))
    meta = dict(g_c=g_c.tolist())
    dense = dict(idx=idx_np, hot=hot_np, xT=xT_np,
                 dinvR=dinvR_np, pool=pool_np)
    return cfg, dense, meta


# ----------------------------------------------------------------------------
# Bass program
# ----------------------------------------------------------------------------
def build(cfg, repeat=1, stage=99):
    from concourse import bass, bacc, tile, mybir
    from concourse.masks import make_identity

    NSH, GSH, NPCH = cfg["NSH"], cfg["GSH"], cfg["NPCH"]
    NWIN, NCH, N = cfg["NWIN"], cfg["NCH"], cfg["N"]
    chunk_win = cfg["chunk_win"]
    dt = mybir.dt
    AT = mybir.ActivationFunctionType
    OP = mybir.AluOpType
    AX = mybir.AxisListType

    nc = bacc.Bacc("TRN2", target_bir_lowering=False, debug=False,
                   enable_asserts=False, num_devices=NCORES)

    def din(name, shape, d):
        return nc.dram_tensor(name, shape, d, kind="ExternalInput").ap()

    xT_in = din("xT", [F, NSH], dt.bfloat16)
    dinvR_in = din("dinvR", [F, NSH], dt.bfloat16)
    idx_in = din("idx", [128, NCH], dt.int32)
    hot_in = din("hot", [128, NCH * WIN], dt.bfloat16)
    pool_in = din("pool", [128, NPCH * GSH], dt.bfloat16)
    W_in = [din(f"W{l}", [F, F], dt.bfloat16) for l in range(3)]
    gT_in = [din(f"gT{l}", [F, 1], dt.float32) for l in range(2)]
    beT_in = [din(f"beT{l}", [F, 1], dt.float32) for l in range(2)]
    g2r_in = din("g2r", [1, F], dt.float32)
    be2r_in = din("be2r", [1, F], dt.float32)
    fcW_in = din("fcW", [F, C], dt.bfloat16)
    fcbR_in = din("fcbR", [128, C], dt.float32)
    out_t = nc.dram_tensor("out", [GSH, C], dt.float32,
                           kind="ExternalOutput").ap()

    invN = 1.0 / float(N)

    with tile.TileContext(nc) as tc:
        with tc.tile_pool(name="const", bufs=1) as constp, \
             tc.tile_pool(name="big1", bufs=1) as big1, \
             tc.tile_pool(name="big2", bufs=2) as big2, \
             tc.tile_pool(name="small", bufs=2) as small, \
             tc.tile_pool(name="msgs", bufs=12) as msgp, \
             tc.tile_pool(name="hT", bufs=2) as hTp, \
             tc.tile_pool(name="psA", bufs=2, space="PSUM") as psA, \
             tc.tile_pool(name="psB", bufs=2, space="PSUM") as psB, \
             tc.tile_pool(name="psC", bufs=2, space="PSUM") as psC, \
             tc.tile_pool(name="dram", bufs=2, space="DRAM") as dram:

            # ---- resident constants ----
            idx_t = constp.tile([128, NCH], dt.int32)
            nc.sync.dma_start(out=idx_t[:], in_=idx_in[:])
            hot_t = constp.tile([128, NCH, WIN], dt.bfloat16)
            nc.sync.dma_start(out=hot_t[:],
                              in_=hot_in[:].rearrange("p (c m) -> p c m", m=WIN))
            pool_t = constp.tile([128, NPCH, GSH], dt.bfloat16)
            nc.sync.dma_start(out=pool_t[:],
                              in_=pool_in[:].rearrange("p (c m) -> p c m", m=GSH))
            W_t = []
            for l in range(3):
                w = constp.tile([F, F], dt.bfloat16, tag=f"W{l}")
                nc.sync.dma_start(out=w[:], in_=W_in[l][:])
                W_t.append(w)
            gT_t, beT_t = [], []
            for l in range(2):
                g = constp.tile([F, 1], dt.float32, tag=f"g{l}")
                nc.sync.dma_start(out=g[:], in_=gT_in[l][:])
                gT_t.append(g)
                b = constp.tile([F, 1], dt.float32, tag=f"be{l}")
                nc.sync.dma_start(out=b[:], in_=beT_in[l][:])
                beT_t.append(b)
            g2r_t = constp.tile([1, F], dt.float32, tag="g2r")
            nc.sync.dma_start(out=g2r_t[:], in_=g2r_in[:])
            be2r_t = constp.tile([1, F], dt.float32, tag="be2r")
            nc.sync.dma_start(out=be2r_t[:], in_=be2r_in[:])
            fcW_t = constp.tile([F, C], dt.bfloat16, tag="fcW")
            nc.sync.dma_start(out=fcW_t[:], in_=fcW_in[:])
            fcbR_t = constp.tile([128, C], dt.float32, tag="fcbR")
            nc.sync.dma_start(out=fcbR_t[:], in_=fcbR_in[:])
            dinvR_t = constp.tile([F, NSH], dt.bfloat16, tag="dinvR")
            nc.sync.dma_start(out=dinvR_t[:], in_=dinvR_in[:])
            xT_t = constp.tile([F, NSH], dt.bfloat16, tag="xT")
            nc.sync.dma_start(out=xT_t[:], in_=xT_in[:])
            ident = constp.tile([128, 128], dt.float32, tag="ident")
            make_identity(nc, ident[:])
            identb = constp.tile([128, 128], dt.bfloat16, tag="identb")
            nc.vector.tensor_copy(identb[:], ident[:])
            ones = constp.tile([128, 1], dt.float32, tag="ones")
            nc.vector.memset(ones[:], 1.0)
            onesr = constp.tile([1, 128], dt.float32, tag="onesr")
            nc.vector.memset(onesr[:], 1.0)

            for _ in range(repeat):
                hT_cur = xT_t          # [F, NSH] bf16, dinv pre-folded
                for l in range(3):
                    if stage < 1 or (l > 0 and stage < 6):
                        break
                    # ---- z = hT.T @ W  (node-major chunks) ----
                    z_sh = big2.tile([128, NPCH, F], dt.bfloat16, tag="z_sh")
                    for j in range(NPCH):
                        pz = psC.tile([128, F], dt.float32, tag="pz")
                        nc.tensor.matmul(pz[:],
                                         lhsT=hT_cur[:, j * 128:(j + 1) * 128],
                                         rhs=W_t[l][:], start=True, stop=True)
                        nc.vector.tensor_copy(z_sh[:, j, :], pz[:])
                    # z_sh[p, j, :] is node slot j*128+p -> DRAM row-major
                    if stage < 2:
                        continue
                    cc_in = dram.tile([NSH, F], dt.bfloat16, tag="cc_in")
                    nc.sync.dma_start(
                        out=cc_in[:].rearrange("(a p) f -> p a f", p=128),
                        in_=z_sh[:])
                    z_full = dram.tile([NCORES * NSH, F], dt.bfloat16,
                                       tag="z_full")
                    nc.gpsimd.collective_compute(
                        "AllGather", mybir.AluOpType.bypass,
                        replica_groups=[list(range(NCORES))],
                        ins=[cc_in.opt()], outs=[z_full.opt()])
                    zf32 = z_full[:].bitcast(dt.float32)
                    if stage < 3:
                        continue

                    # ---- gather + one-hot scatter matmuls into PSUM ----
                    agg = big2.tile([128, NPCH, F], dt.float32, tag="agg")
                    pw = None
                    for ch in range(NCH):
                        w = chunk_win[ch]
                        wo = w % 2          # window within [64, F] psum pair
                        first = ch == 0 or chunk_win[ch - 1] != w
                        last = ch == NCH - 1 or chunk_win[ch + 1] != w
                        if wo == 0 and first and stage >= 4:
                            pw = psA.tile([64, F], dt.float32, tag="pw")
                        m = msgp.tile([128, F // 2], dt.float32, tag="m")
                        nc.gpsimd.indirect_dma_start(
                            out=m[:], out_offset=None, in_=zf32,
                            in_offset=bass.IndirectOffsetOnAxis(
                                ap=idx_t[:, ch:ch + 1], axis=0))
                        if stage < 4:
                            continue
                        nc.tensor.matmul(
                            pw[wo * WIN:(wo + 1) * WIN, :],
                            lhsT=hot_t[:, ch, :],
                            rhs=m[:].bitcast(dt.bfloat16),
                            start=first, stop=last)
                        if last and (wo == 1 or w == NWIN - 1):
                            j, half = w // 4, (w % 4) // 2
                            nc.vector.tensor_copy(
                                agg[half * 64:(half + 1) * 64, j, :], pw[:])

                    if stage < 5:
                        continue
                    if l < 2:
                        # ---- transposed BN: aggT, stats, relu, dinv ----
                        aggT = big1.tile([F, NSH], dt.float32, tag="aggT")
                        sums = small.tile([F, NPCH], dt.float32, tag="sums")
                        sqs = small.tile([F, NPCH], dt.float32, tag="sqs")
                        for j in range(NPCH):
                            sl = slice(j * 128, (j + 1) * 128)
                            pt = psB.tile([F, 128], dt.float32, tag="pt")
                            nc.tensor.transpose(pt[:], in_=agg[:, j, :],
                                                identity=ident[:])
                            nc.scalar.activation(
                                aggT[:, sl], pt[:], AT.Copy,
                                accum_out=sums[:, j:j + 1])
                            scr = small.tile([F, 128], dt.float32, tag="scr")
                            nc.scalar.activation(scr[:], aggT[:, sl],
                                                 AT.Square,
                                                 accum_out=sqs[:, j:j + 1])
                        st = small.tile([F, 2], dt.float32, tag="st")
                        nc.vector.tensor_reduce(st[:, 0:1], sums[:],
                                                axis=AX.X, op=OP.add)
                        nc.vector.tensor_reduce(st[:, 1:2], sqs[:],
                                                axis=AX.X, op=OP.add)
                        ccs_in = dram.tile([F, 2], dt.float32, tag="ccs_in")
                        ccs_out = dram.tile([F, 2], dt.float32, tag="ccs_out")
                        nc.sync.dma_start(out=ccs_in[:], in_=st[:])
                        nc.gpsimd.collective_compute(
                            "AllReduce", mybir.AluOpType.add,
                            replica_groups=[list(range(NCORES))],
                            ins=[ccs_in.opt()], outs=[ccs_out.opt()])
                        stg = small.tile([F, 2], dt.float32, tag="stg")
                        nc.sync.dma_start(out=stg[:], in_=ccs_out[:])
                        mu = small.tile([F, 1], dt.float32, tag="mu")
                        nc.vector.tensor_scalar_mul(mu[:], stg[:, 0:1], invN)
                        var = small.tile([F, 1], dt.float32, tag="var")
                        nc.vector.tensor_scalar_mul(var[:], stg[:, 1:2], invN)
                        mu2 = small.tile([F, 1], dt.float32, tag="mu2")
                        nc.vector.tensor_tensor(mu2[:], mu[:], mu[:],
                                                op=OP.mult)
                        nc.vector.tensor_tensor(var[:], var[:], mu2[:],
                                                op=OP.subtract)
                        nc.vector.tensor_scalar_add(var[:], var[:], EPS)
                        rst = small.tile([F, 1], dt.float32, tag="rst")
                        nc.scalar.sqrt(rst[:], var[:])
                        nc.vector.reciprocal(rst[:], rst[:])
                        s_t = small.tile([F, 1], dt.float32, tag="s_t")
                        nc.vector.tensor_tensor(s_t[:], rst[:], gT_t[l][:],
                                                op=OP.mult)
                        t_t = small.tile([F, 1], dt.float32, tag="t_t")
                        nc.vector.tensor_tensor(t_t[:], mu[:], s_t[:],
                                                op=OP.mult)
                        nc.vector.tensor_tensor(t_t[:], beT_t[l][:], t_t[:],
                                                op=OP.subtract)
                        hT_new = hTp.tile([F, NSH], dt.bfloat16, tag="hT")
                        for j in range(NPCH):
                            sl = slice(j * 128, (j + 1) * 128)
                            nc.scalar.activation(
                                hT_new[:, sl], aggT[:, sl], AT.Relu,
                                bias=t_t[:, 0:1], scale=s_t[:, 0:1])
                        nc.vector.tensor_tensor(hT_new[:], hT_new[:],
                                                dinvR_t[:], op=OP.mult)
                        hT_cur = hT_new
                    else:
                        # ---- node-major BN (last conv) + pooling + fc ----
                        ps2 = psB.tile([1, F], dt.float32, tag="pt")
                        for j in range(NPCH):
                            sq = small.tile([128, F], dt.float32, tag="sq")
                            nc.vector.tensor_tensor(sq[:], agg[:, j, :],
                                                    agg[:, j, :], op=OP.mult)
                            nc.tensor.matmul(ps2[:], lhsT=ones[:],
                                             rhs=sq[:],
                                             start=(j == 0),
                                             stop=(j == NPCH - 1))
                        st2 = small.tile([1, 2 * F], dt.float32, tag="st2")
                        nc.vector.tensor_copy(st2[:, F:2 * F], ps2[:])
                        ps1 = psB.tile([1, F], dt.float32, tag="pt")
                        for j in range(NPCH):
                            nc.tensor.matmul(ps1[:], lhsT=ones[:],
                                             rhs=agg[:, j, :],
                                             start=(j == 0),
                                             stop=(j == NPCH - 1))
                        nc.vector.tensor_copy(st2[:, 0:F], ps1[:])
                        ccs2_in = dram.tile([1, 2 * F], dt.float32, tag="cc2i")
                        ccs2_out = dram.tile([1, 2 * F], dt.float32, tag="cc2o")
                        nc.sync.dma_start(out=ccs2_in[:], in_=st2[:])
                        nc.gpsimd.collective_compute(
                            "AllReduce", mybir.AluOpType.add,
                            replica_groups=[list(range(NCORES))],
                            ins=[ccs2_in.opt()], outs=[ccs2_out.opt()])
                        stg2 = small.tile([1, 2 * F], dt.float32, tag="stg2")
                        nc.sync.dma_start(out=stg2[:], in_=ccs2_out[:])
                        mu_r = small.tile([1, F], dt.float32, tag="mu_r")
                        nc.vector.tensor_scalar_mul(mu_r[:], stg2[:, 0:F],
                                                    invN)
                        var_r = small.tile([1, F], dt.float32, tag="var_r")
                        nc.vector.tensor_scalar_mul(var_r[:], stg2[:, F:2 * F],
                                                    invN)
                        mu2_r = small.tile([1, F], dt.float32, tag="mu2_r")
                        nc.vector.tensor_tensor(mu2_r[:], mu_r[:], mu_r[:],
                                                op=OP.mult)
                        nc.vector.tensor_tensor(var_r[:], var_r[:], mu2_r[:],
                                                op=OP.subtract)
                        nc.vector.tensor_scalar_add(var_r[:], var_r[:], EPS)
                        rst_r = small.tile([1, F], dt.float32, tag="rst_r")
                        nc.scalar.sqrt(rst_r[:], var_r[:])
                        nc.vector.reciprocal(rst_r[:], rst_r[:])
                        s_r = small.tile([1, F], dt.float32, tag="s_r")
                        nc.vector.tensor_tensor(s_r[:], rst_r[:], g2r_t[:],
                                                op=OP.mult)
                        t_r = small.tile([1, F], dt.float32, tag="t_r")
                        nc.vector.tensor_tensor(t_r[:], mu_r[:], s_r[:],
                                                op=OP.mult)
                        nc.vector.tensor_tensor(t_r[:], be2r_t[:], t_r[:],
                                                op=OP.subtract)
                        # broadcast s,t across partitions via PE outer product
                        psb = psB.tile([128, F], dt.float32, tag="pt")
                        nc.tensor.matmul(psb[:], lhsT=onesr[:], rhs=s_r[:],
                                         start=True, stop=True)
                        s_rep = small.tile([128, F], dt.float32, tag="s_rep")
                        nc.vector.tensor_copy(s_rep[:], psb[:])
                        psb2 = psB.tile([128, F], dt.float32, tag="pt")
                        nc.tensor.matmul(psb2[:], lhsT=onesr[:], rhs=t_r[:],
                                         start=True, stop=True)
                        t_rep = small.tile([128, F], dt.float32, tag="t_rep")
                        nc.vector.tensor_copy(t_rep[:], psb2[:])
                        h3 = big2.tile([128, NPCH, F], dt.bfloat16, tag="z_sh")
                        for j in range(NPCH):
                            tmp = small.tile([128, F], dt.float32, tag="tmp")
                            nc.vector.tensor_tensor(tmp[:], agg[:, j, :],
                                                    s_rep[:], op=OP.mult)
                            nc.vector.tensor_tensor(tmp[:], tmp[:], t_rep[:],
                                                    op=OP.add)
                            nc.vector.tensor_scalar_max(h3[:, j, :], tmp[:],
                                                        0.0)
                        # pooling
                        pp = psB.tile([GSH, F], dt.float32, tag="pt")
                        for j in range(NPCH):
                            nc.tensor.matmul(pp[:], lhsT=pool_t[:, j, :],
                                             rhs=h3[:, j, :],
                                             start=(j == 0),
                                             stop=(j == NPCH - 1))
                        pooled = small.tile([GSH, F], dt.bfloat16,
                                            tag="pooled")
                        nc.vector.tensor_copy(pooled[:], pp[:])
                        ppT = psB.tile([F, GSH], dt.bfloat16, tag="pt")
                        nc.tensor.transpose(ppT[:], in_=pooled[:],
                                            identity=identb[:GSH, :GSH])
                        pooledT = small.tile([F, GSH], dt.bfloat16,
                                             tag="pooledT")
                        nc.vector.tensor_copy(pooledT[:], ppT[:])
                        pfc = psB.tile([GSH, C], dt.float32, tag="pt")
                        nc.tensor.matmul(pfc[:], lhsT=pooledT[:],
                                         rhs=fcW_t[:], start=True, stop=True)
                        res = small.tile([GSH, C], dt.float32, tag="res")
                        nc.vector.tensor_tensor(res[:], pfc[:],
                                                fcbR_t[:GSH, :], op=OP.add)
                        nc.sync.dma_start(out=out_t[:], in_=res[:])
    nc.compile()
    return nc


# ----------------------------------------------------------------------------
# Execution (PJRT via axon)
# ----------------------------------------------------------------------------
class BassExec:
    def __init__(self, nc, n_cores):
        import jax
        from jax.sharding import Mesh, PartitionSpec, NamedSharding
        from jax.experimental.shard_map import shard_map
        from concourse import mybir
        from concourse.bass2jax import (
            install_neuronx_cc_hook, _bass_exec_p, partition_id_tensor)
        install_neuronx_cc_hook()
        self.jax = jax
        self.n_cores = n_cores
        partition_name = (nc.partition_id_tensor.name
                          if nc.partition_id_tensor else None)
        in_names, out_names, out_avals, zero_outs = [], [], [], []
        for alloc in nc.m.functions[0].allocations:
            if not isinstance(alloc, mybir.MemoryLocationSet):
                continue
            name = alloc.memorylocations[0].name
            if alloc.kind == "ExternalInput":
                if name != partition_name:
                    in_names.append(name)
            elif alloc.kind == "ExternalOutput":
                shape = tuple(alloc.tensor_shape)
                dtype = mybir.dt.np(alloc.dtype)
                out_names.append(name)
                out_avals.append(jax.core.ShapedArray(shape, dtype))
                zero_outs.append(np.zeros(shape, dtype))
        self.in_names, self.out_names = in_names, out_names
        self.out_avals, self.zero_outs = out_avals, zero_outs
        n_params, n_outs = len(in_names), len(out_avals)
        all_in = in_names + out_names + (
            [partition_name] if partition_name else [])

        def _body(*args):
            operands = list(args)
            if partition_name is not None:
                operands.append(partition_id_tensor())
            outs = _bass_exec_p.bind(
                *operands, out_avals=tuple(out_avals),
                in_names=tuple(all_in), out_names=tuple(out_names),
                lowering_input_output_aliases=(),
                sim_require_finite=True, sim_require_nnan=True, nc=nc)
            return tuple(outs)

        try:
            devices = jax.devices("axon")[:n_cores]
        except RuntimeError:
            devices = jax.devices()[:n_cores]
        self.mesh = Mesh(np.asarray(devices), ("core",))
        in_specs = (PartitionSpec("core"),) * (n_params + n_outs)
        out_specs = (PartitionSpec("core"),) * n_outs
        self.fn = jax.jit(
            shard_map(_body, mesh=self.mesh, in_specs=in_specs,
                      out_specs=out_specs, check_rep=False),
            donate_argnums=tuple(range(n_params, n_params + n_outs)),
            keep_unused=True)
        self.sharding = NamedSharding(self.mesh, PartitionSpec("core"))
        self._dev_in = None

    def put_inputs(self, in_maps):
        concat = [np.concatenate([np.ascontiguousarray(in_maps[c][n])
                                  for c in range(self.n_cores)], axis=0)
                  for n in self.in_names]
        self._dev_in = [self.jax.device_put(a, self.sharding) for a in concat]
        self.jax.block_until_ready(self._dev_in)

    def run(self):
        zs = [self.jax.device_put(
            np.zeros((self.n_cores * z.shape[0], *z.shape[1:]), z.dtype),
            self.sharding) for z in self.zero_outs]
        self.jax.block_until_ready(zs)
        outs = self.fn(*self._dev_in, *zs)
        self.jax.block_until_ready(outs)
        return [
            {n: np.asarray(outs[i]).reshape(
                self.n_cores, *self.out_avals[i].shape)[c]
             for i, n in enumerate(self.out_names)}
            for c in range(self.n_cores)
        ]


_CACHE = {}


def _get_exec(cfg, repeat=1):
    key = (cfg["NSH"], cfg["GSH"], cfg["NCH"], cfg["N"],
           cfg["chunk_win"], repeat)
    if key not in _CACHE:
        nc = build(cfg, repeat=repeat)
        _CACHE[key] = BassExec(nc, NCORES)
    return _CACHE[key]


def make_core_inputs(dense, Ws, gs, bes, fcW, fcb, cfg):
    GSH, NPCH, NCH = cfg["GSH"], cfg["NPCH"], cfg["NCH"]
    ins = []
    fcb_f = np.asarray(fcb, np.float32).reshape(1, C)
    for c in range(NCORES):
        m = {
            "xT": dense["xT"][c],
            "dinvR": dense["dinvR"][c],
            "idx": dense["idx"][c],
            "hot": dense["hot"][c].reshape(128, NCH * WIN),
            "pool": dense["pool"][c].reshape(128, NPCH * GSH),
            "g2r": np.asarray(gs[2], np.float32).reshape(1, F),
            "be2r": np.asarray(bes[2], np.float32).reshape(1, F),
            "fcW": np.asarray(fcW, np.float32).astype(BF16),
            "fcbR": np.broadcast_to(fcb_f, (128, C)).copy(),
        }
        for l in range(3):
            m[f"W{l}"] = np.asarray(Ws[l], np.float32).astype(BF16)
        for l in range(2):
            m[f"gT{l}"] = np.asarray(gs[l], np.float32).reshape(F, 1)
            m[f"beT{l}"] = np.asarray(bes[l], np.float32).reshape(F, 1)
        ins.append(m)
    return ins


def kernel(x, edge_index, batch, W0, b0, g0, be0, W1, b1, g1, be1,
           W2, b2, g2, be2, fcW, fcb):
    # note: conv bias b_l is mathematically cancelled by BatchNorm centering.
    G = 500
    cfg, dense, meta = prep(np.asarray(x), np.asarray(edge_index),
                            np.asarray(batch), G)
    ex = _get_exec(cfg)
    ins = make_core_inputs(dense, [W0, W1, W2], [g0, g1, g2],
                           [be0, be1, be2], fcW, fcb, cfg)
    ex.put_inputs(ins)
    res = ex.run()
    parts = [res[c]["out"][:meta["g_c"][c]] for c in range(NCORES)]
    return np.concatenate(parts, axis=0).astype(np.float32)
